# revision 2
# baseline (speedup 1.0000x reference)
"""Trainium2 Bass kernel for nn_AttentionHead (sparse/locally-connected attention).

Computation (per batch b):
    q = x @ (Wl*mask @ Wq*mask).T + (Wl*mask) @ bq        [S, H]
    k = x @ (Wk*mask).T + bk                              [S, H]
    v = x @ (Wv*mask).T + bv                              [S, H]
    scores = q @ k.T / sqrt(H)                            [S, S]
    probs  = softmax(scores, axis=-1)
    out    = probs @ v                                    [S, H]

Sharding: data-parallel over batch — core b computes batch b entirely
(weights replicated, no collectives).

On-core dataflow (all matmuls bf16 inputs, fp32 PSUM accumulate):
    xT   <- DMA-xbar-transpose(x)                 [h-part, s-free]
    qT,kT <- W.T-stationary matmuls over xT       [h'-part, s-free]
    v    <- xT-stationary matmuls                 [t-part, h-free]
    per 128-row block m:
        scores -> PSUM, ACT exp(+rowsum) -> E (bf16)
        probs  = E * (1/Z)  (DVE, per-partition scalar) -> DRAM
        ET     <- one DMA-xbar-transpose of E
        out    = (ET.T @ v) * (1/Z) -> DRAM
"""

import math

import ml_dtypes
import numpy as np

import concourse.bass as bass
import concourse.mybir as mybir
import concourse.tile as tile
from concourse import bacc, bass_utils

BF16 = ml_dtypes.bfloat16

B, S, H = 8, 2048, 1024
SQ = 5
P = 128
KT = H // P        # 8 contraction tiles over h
ST = S // P        # 16 sequence blocks
NCH = S // 512     # 4 512-chunks over s/t
HCH = H // 512     # 2 512-chunks over h
N_CORES = 8

_cache = {}


def _locality_mask(hidden_size: int, width: int) -> np.ndarray:
    side = int(round(math.sqrt(hidden_size)))
    assert side * side == hidden_size
    r = np.arange(hidden_size) // side
    c = np.arange(hidden_size) % side
    dr = np.abs(r[:, None] - r[None, :])
    dc = np.abs(c[:, None] - c[None, :])
    dr = np.minimum(dr, side - dr)
    dc = np.minimum(dc, side - dc)
    half = width // 2
    return ((dr <= half) & (dc <= half)).astype(np.float32)


def _build_program():
    f32 = mybir.dt.float32
    bf = mybir.dt.bfloat16
    PSUM = bass.MemorySpace.PSUM
    Ident = mybir.ActivationFunctionType.Identity
    Exp = mybir.ActivationFunctionType.Exp

    nc = bacc.Bacc("TRN2", target_bir_lowering=False, debug=False)

    x_d = nc.dram_tensor("x", [S, H], bf, kind="ExternalInput")
    wq_d = nc.dram_tensor("wqT", [H, H], bf, kind="ExternalInput")
    wk_d = nc.dram_tensor("wkT", [H, H], bf, kind="ExternalInput")
    wv_d = nc.dram_tensor("wvT", [H, H], bf, kind="ExternalInput")
    bq_d = nc.dram_tensor("bq", [P, KT], f32, kind="ExternalInput")
    bk_d = nc.dram_tensor("bk", [P, KT], f32, kind="ExternalInput")
    bv_d = nc.dram_tensor("bv", [H], f32, kind="ExternalInput")
    out_d = nc.dram_tensor("out", [S, H], f32, kind="ExternalOutput")
    probs_d = nc.dram_tensor("probs", [S, S], f32, kind="ExternalOutput")

    with tile.TileContext(nc) as tc:
        with tc.tile_pool(name="persist", bufs=1) as persist:
            qT = [persist.tile([P, S], bf, tag=f"qT{k}", name=f"qT{k}") for k in range(KT)]
            kTt = [persist.tile([P, S], bf, tag=f"kT{k}", name=f"kT{k}") for k in range(KT)]
            vt = [persist.tile([P, H], bf, tag=f"v{i}", name=f"v{i}") for i in range(ST)]

            # ---------------- phase 1: projections ----------------
            with (
                tc.tile_pool(name="proj", bufs=1) as proj,
                tc.tile_pool(name="pp", bufs=2, space=PSUM) as pp,
            ):
                wq_sb = [proj.tile([P, H], bf, tag=f"wq{k}", name=f"wq{k}") for k in range(KT)]
                wk_sb = [proj.tile([P, H], bf, tag=f"wk{k}", name=f"wk{k}") for k in range(KT)]
                wv_sb = [proj.tile([P, H], bf, tag=f"wv{k}", name=f"wv{k}") for k in range(KT)]
                xT = [proj.tile([P, S], bf, tag=f"xT{k}", name=f"xT{k}") for k in range(KT)]
                bq_sb = proj.tile([P, KT], f32, tag="bq")
                bk_sb = proj.tile([P, KT], f32, tag="bk")
                bv_sb = proj.tile([P, H], f32, tag="bv")

                for k in range(KT):
                    ks = slice(k * P, (k + 1) * P)
                    nc.gpsimd.dma_start(out=wq_sb[k][:], in_=wq_d.ap()[ks, :])
                    nc.gpsimd.dma_start(out=wk_sb[k][:], in_=wk_d.ap()[ks, :])
                    nc.gpsimd.dma_start(out=wv_sb[k][:], in_=wv_d.ap()[ks, :])
                    nc.sync.dma_start(out=xT[k][:], in_=x_d.ap()[:, ks], transpose=True)
                nc.gpsimd.dma_start(out=bq_sb[:], in_=bq_d.ap())
                nc.gpsimd.dma_start(out=bk_sb[:], in_=bk_d.ap())
                bv_ap = bv_d.ap()
                bv_bcast = bass.AP(
                    tensor=bv_ap.tensor, offset=bv_ap.offset,
                    ap=[[0, P]] + list(bv_ap.ap),
                )
                nc.gpsimd.dma_start(out=bv_sb[:], in_=bv_bcast)

                # kT then qT: out[h'-tile m, s] = sum_h W.T[h, m-slice].T @ xT[h, s]
                for w_sb, b_sb, dstT in ((wk_sb, bk_sb, kTt), (wq_sb, bq_sb, qT)):
                    for m in range(KT):
                        ps = pp.tile([P, S], f32, tag="pp", name="pps")
                        for j in range(NCH):
                            js = slice(j * 512, (j + 1) * 512)
                            for k in range(KT):
                                nc.tensor.matmul(
                                    ps[:, js],
                                    lhsT=w_sb[k][:, m * P:(m + 1) * P],
                                    rhs=xT[k][:, js],
                                    start=(k == 0), stop=(k == KT - 1),
                                )
                            nc.scalar.activation(
                                dstT[m][:, js], ps[:, js], Ident,
                                bias=b_sb[:, m:m + 1],
                            )

                # v: out[t-tile i, h] = sum_h xT[h, i-slice].T @ WvT[h, :]
                for i in range(ST):
                    psv = pp.tile([P, H], f32, tag="pp", name="ppv")
                    for j in range(HCH):
                        js = slice(j * 512, (j + 1) * 512)
                        for k in range(KT):
                            nc.tensor.matmul(
                                psv[:, js],
                                lhsT=xT[k][:, i * P:(i + 1) * P],
                                rhs=wv_sb[k][:, js],
                                start=(k == 0), stop=(k == KT - 1),
                            )
                    nc.vector.tensor_add(vt[i][:], psv[:], bv_sb[:])

            # ---------------- phase 2: attention ----------------
            with (
                tc.tile_pool(name="aE", bufs=3) as aE,
                tc.tile_pool(name="a2", bufs=2) as a2,
                tc.tile_pool(name="stats", bufs=4) as stats,
                tc.tile_pool(name="scp", bufs=6, space=PSUM) as scp,
                tc.tile_pool(name="opp", bufs=2, space=PSUM) as opp,
            ):
                inv_sqrt_h = float(1.0 / math.sqrt(H))
                for m in range(ST):
                    ms = slice(m * P, (m + 1) * P)
                    E = aE.tile([P, S], bf, tag="E", name="E")
                    zacc = stats.tile([P, NCH], f32, tag="zacc", name="zacc")
                    for j in range(NCH):
                        js = slice(j * 512, (j + 1) * 512)
                        sc = scp.tile([P, 512], f32, tag="sc", name="sc")
                        for k in range(KT):
                            nc.tensor.matmul(
                                sc[:],
                                lhsT=qT[k][:, ms],
                                rhs=kTt[k][:, js],
                                start=(k == 0), stop=(k == KT - 1),
                            )
                        nc.scalar.activation(
                            E[:, js], sc[:], Exp,
                            scale=inv_sqrt_h, accum_out=zacc[:, j:j + 1],
                        )
                    z = stats.tile([P, 1], f32, tag="z", name="z")
                    nc.vector.reduce_sum(z[:], zacc[:], axis=mybir.AxisListType.X)
                    r = stats.tile([P, 1], f32, tag="r", name="r")
                    nc.vector.reciprocal(r[:], z[:])

                    pr = a2.tile([P, S], f32, tag="pr", name="pr")
                    nc.vector.tensor_scalar_mul(pr[:], E[:], r[:])
                    nc.gpsimd.dma_start(out=probs_d.ap()[ms, :], in_=pr[:])

                    ET = a2.tile([P, ST, P], bf, tag="ET", name="ET")
                    nc.sync.dma_start(out=ET[:], in_=E[:], transpose=True)

                    ot = a2.tile([P, H], f32, tag="ot", name="ot")
                    for j in range(HCH):
                        js = slice(j * 512, (j + 1) * 512)
                        op = opp.tile([P, 512], f32, tag="op", name="op")
                        for k2 in range(ST):
                            nc.tensor.matmul(
                                op[:],
                                lhsT=ET[:, k2, :],
                                rhs=vt[k2][:, js],
                                start=(k2 == 0), stop=(k2 == ST - 1),
                            )
                        nc.vector.tensor_scalar_mul(ot[:, js], op[:], r[:])
                    nc.gpsimd.dma_start(out=out_d.ap()[ms, :], in_=ot[:])

    nc.compile()
    return nc


def _prep_shared(Wq, bq, Wk, bk, Wv, bv, Wl):
    mask = _locality_mask(H, SQ)
    Wqm = Wq.astype(np.float32) * mask
    Wkm = Wk.astype(np.float32) * mask
    Wvm = Wv.astype(np.float32) * mask
    Wlm = Wl.astype(np.float32) * mask
    Wq_eff = Wlm @ Wqm
    bq_eff = Wlm @ bq.astype(np.float32)
    return {
        "wqT": np.ascontiguousarray(Wq_eff.T).astype(BF16),
        "wkT": np.ascontiguousarray(Wkm.T).astype(BF16),
        "wvT": np.ascontiguousarray(Wvm.T).astype(BF16),
        "bq": np.ascontiguousarray(bq_eff.reshape(KT, P).T).astype(np.float32),
        "bk": np.ascontiguousarray(bk.astype(np.float32).reshape(KT, P).T),
        "bv": bv.astype(np.float32),
    }


def _make_in_maps(inputs):
    x = np.asarray(inputs["x"])
    shared = _prep_shared(
        np.asarray(inputs["Wq"]), np.asarray(inputs["bq"]),
        np.asarray(inputs["Wk"]), np.asarray(inputs["bk"]),
        np.asarray(inputs["Wv"]), np.asarray(inputs["bv"]),
        np.asarray(inputs["Wl"]),
    )
    in_maps = []
    for b in range(N_CORES):
        m = dict(shared)
        m["x"] = np.ascontiguousarray(x[b]).astype(BF16)
        in_maps.append(m)
    return in_maps


def _get_program():
    nc = _cache.get("nc")
    if nc is None:
        nc = _build_program()
        _cache["nc"] = nc
    return nc


def _run(inputs, trace=False, tmpdir=None):
    nc = _get_program()
    in_maps = _make_in_maps(inputs)
    res = bass_utils.run_bass_kernel_spmd(
        nc, in_maps, core_ids=list(range(N_CORES)), trace=trace, tmpdir=tmpdir,
    )
    out = np.stack([res.results[b]["out"] for b in range(N_CORES)])
    probs = np.stack([res.results[b]["probs"] for b in range(N_CORES)])
    return (out, probs), res


def kernel(**inputs):
    (out, probs), _ = _run(inputs)
    return out, probs


# revision 4
# speedup vs baseline: 1.2829x; 1.2829x over previous
"""Trainium2 Bass kernel for nn_AttentionHead (sparse/locally-connected attention).

Computation (per batch b):
    q = x @ (Wl*mask @ Wq*mask).T + (Wl*mask) @ bq        [S, H]
    k = x @ (Wk*mask).T + bk                              [S, H]
    v = x @ (Wv*mask).T + bv                              [S, H]
    scores = q @ k.T / sqrt(H)                            [S, S]
    probs  = softmax(scores, axis=-1)
    out    = probs @ v                                    [S, H]

Sharding: data-parallel over batch — core b computes batch b entirely
(weights replicated, no collectives).

The locality mask couples only units within Chebyshev distance 2 on a
32x32 grid (wrap-around), so at 128-row tile granularity (4 grid rows)
every masked weight matrix is block-tridiagonal (circulant): block
(I, J) is nonzero only for J in {I-1, I, I+1} mod 8. The folded
Wl*mask @ Wq*mask reaches +-4 grid rows = +-1 block, so it is block-
tridiagonal too. Projections therefore skip 5 of 8 contraction blocks.

On-core dataflow (all matmuls bf16 inputs, fp32 PSUM accumulate):
    xT   <- DMA-xbar-transpose(x)                 [h-part, s-free]
    qT,kT <- block-sparse W-stationary matmuls    [h'-part, s-free]
    v    <- xT-stationary block-sparse matmuls    [t-part, h-free]
    per 128-row block m:
        scores -> PSUM, ACT exp(+rowsum) -> E (bf16)
        probs  = E * (1/Z)  (DVE, per-partition scalar) -> DRAM
        ET     <- one DMA-xbar-transpose of E
        out    = (ET.T @ v) * (1/Z) -> DRAM
"""

import math

import ml_dtypes
import numpy as np

import concourse.bass as bass
import concourse.mybir as mybir
import concourse.tile as tile
from concourse import bacc, bass_utils

BF16 = ml_dtypes.bfloat16

B, S, H = 8, 2048, 1024
SQ = 5
P = 128
KT = H // P        # 8 feature tiles
ST = S // P        # 16 sequence blocks
NCH = S // 512     # 4 512-chunks over s/t
HCH = H // 512     # 2 512-chunks over h
N_CORES = 8

_cache = {}


def _locality_mask(hidden_size: int, width: int) -> np.ndarray:
    side = int(round(math.sqrt(hidden_size)))
    assert side * side == hidden_size
    r = np.arange(hidden_size) // side
    c = np.arange(hidden_size) % side
    dr = np.abs(r[:, None] - r[None, :])
    dc = np.abs(c[:, None] - c[None, :])
    dr = np.minimum(dr, side - dr)
    dc = np.minimum(dc, side - dc)
    half = width // 2
    return ((dr <= half) & (dc <= half)).astype(np.float32)


def _block_lists(support: np.ndarray):
    """support: [H, H] bool-ish. Returns blists[i] = sorted js with any
    nonzero in 128-block (i, j)."""
    blk = support.reshape(KT, P, KT, P).any(axis=(1, 3))
    return [sorted(np.nonzero(blk[i])[0].tolist()) for i in range(KT)]


def _mask_supports():
    mask = _locality_mask(H, SQ)
    sup1 = mask > 0                       # support of Wk', Wv' (symmetric)
    sup2 = (mask @ mask) > 0              # support of Wl'@Wq'
    return sup1, sup2


def _build_program():
    f32 = mybir.dt.float32
    bf = mybir.dt.bfloat16
    PSUM = bass.MemorySpace.PSUM
    Ident = mybir.ActivationFunctionType.Identity
    Exp = mybir.ActivationFunctionType.Exp

    sup1, sup2 = _mask_supports()
    # For W.T block (k, m): nonzero iff W[m-block, k-block] nonzero.
    # sup is symmetric so row/col lists coincide; keep general anyway.
    nbr_kv = _block_lists(sup1)   # nbr_kv[k] = m/J blocks coupled to k
    nbr_q = _block_lists(sup2)
    NB1 = max(len(l) for l in nbr_kv)
    NBQ = max(len(l) for l in nbr_q)
    assert all(len(l) == NB1 for l in nbr_kv)
    assert all(len(l) == NBQ for l in nbr_q)

    nc = bacc.Bacc("TRN2", target_bir_lowering=False, debug=False)

    x_d = nc.dram_tensor("x", [S, H], bf, kind="ExternalInput")
    # packed nonzero 128x128 blocks of W.T, per feature tile k
    wq_d = nc.dram_tensor("wqP", [KT, P, NBQ, P], bf, kind="ExternalInput")
    wk_d = nc.dram_tensor("wkP", [KT, P, NB1, P], bf, kind="ExternalInput")
    wv_d = nc.dram_tensor("wvP", [KT, P, NB1, P], bf, kind="ExternalInput")
    bq_d = nc.dram_tensor("bq", [P, KT], f32, kind="ExternalInput")
    bk_d = nc.dram_tensor("bk", [P, KT], f32, kind="ExternalInput")
    bv_d = nc.dram_tensor("bv", [H], f32, kind="ExternalInput")
    out_d = nc.dram_tensor("out", [S, H], f32, kind="ExternalOutput")
    probs_d = nc.dram_tensor("probs", [S, S], f32, kind="ExternalOutput")

    # m-order so each block's xT neighbors are among the earliest loads
    m_order = list(range(1, KT)) + [0]

    with tile.TileContext(nc) as tc:
        with (
            tc.tile_pool(name="sb", bufs=1) as sb,
            tc.tile_pool(name="work", bufs=2) as work,
            tc.tile_pool(name="stats", bufs=4) as stats,
            tc.tile_pool(name="ps", bufs=1, space=PSUM) as psp,
        ):
            # persistent activations
            qT = [sb.tile([P, S], bf, tag=f"qT{k}", name=f"qT{k}") for k in range(KT)]
            kTt = [sb.tile([P, S], bf, tag=f"kT{k}", name=f"kT{k}") for k in range(KT)]
            vt = [sb.tile([P, H], bf, tag=f"v{i}", name=f"v{i}") for i in range(ST)]
            # packed weights + biases + xT
            wq_sb = [sb.tile([P, NBQ, P], bf, tag=f"wq{k}", name=f"wq{k}") for k in range(KT)]
            wk_sb = [sb.tile([P, NB1, P], bf, tag=f"wk{k}", name=f"wk{k}") for k in range(KT)]
            wv_sb = [sb.tile([P, NB1, P], bf, tag=f"wv{k}", name=f"wv{k}") for k in range(KT)]
            xT = [sb.tile([P, S], bf, tag=f"xT{k}", name=f"xT{k}") for k in range(KT)]
            bq_sb = sb.tile([P, KT], f32, tag="bq")
            bk_sb = sb.tile([P, KT], f32, tag="bk")
            bv_sb = sb.tile([P, H], f32, tag="bv")

            for k in range(KT):
                nc.sync.dma_start(out=xT[k][:], in_=x_d.ap()[:, k * P:(k + 1) * P],
                                  transpose=True)
            for k in range(KT):
                nc.gpsimd.dma_start(out=wk_sb[k][:], in_=wk_d.ap()[k])
                nc.gpsimd.dma_start(out=wq_sb[k][:], in_=wq_d.ap()[k])
                nc.gpsimd.dma_start(out=wv_sb[k][:], in_=wv_d.ap()[k])
            nc.gpsimd.dma_start(out=bq_sb[:], in_=bq_d.ap())
            nc.gpsimd.dma_start(out=bk_sb[:], in_=bk_d.ap())
            bv_ap = bv_d.ap()
            bv_bcast = bass.AP(tensor=bv_ap.tensor, offset=bv_ap.offset,
                               ap=[[0, P]] + list(bv_ap.ap))
            nc.gpsimd.dma_start(out=bv_sb[:], in_=bv_bcast)

            # ---- kT, qT: block-sparse, weight-stationary ----
            for w_sb, b_sb, dstT, nbr in (
                (wk_sb, bk_sb, kTt, nbr_kv),
                (wq_sb, bq_sb, qT, nbr_q),
            ):
                for m in m_order:
                    klist = [k for k in range(KT) if m in nbr[k]]
                    ps = [psp.tile([P, 512], f32, tag="ps", name="ps", bufs=6)
                          for _ in range(NCH)]
                    for ki, k in enumerate(klist):
                        bidx = nbr[k].index(m)
                        for j in range(NCH):
                            nc.tensor.matmul(
                                ps[j][:],
                                lhsT=w_sb[k][:, bidx, :],
                                rhs=xT[k][:, j * 512:(j + 1) * 512],
                                start=(ki == 0), stop=(ki == len(klist) - 1),
                            )
                    for j in range(NCH):
                        nc.scalar.activation(
                            dstT[m][:, j * 512:(j + 1) * 512], ps[j][:], Ident,
                            bias=b_sb[:, m:m + 1],
                        )

            # ---- v: xT-stationary, block-sparse over output chunks ----
            for i in range(ST):
                psv = [psp.tile([P, 512], f32, tag="ps", name="psv", bufs=6)
                       for _ in range(HCH)]
                for J in range(KT):
                    contribs = [kk for kk in range(KT) if J in nbr_kv[kk]]
                    for ci, k in enumerate(contribs):
                        bidx = nbr_kv[k].index(J)
                        nc.tensor.matmul(
                            psv[J // 4][:, (J % 4) * P:(J % 4 + 1) * P],
                            lhsT=xT[k][:, i * P:(i + 1) * P],
                            rhs=wv_sb[k][:, bidx, :],
                            start=(ci == 0), stop=(ci == len(contribs) - 1),
                        )
                for j in range(HCH):
                    nc.vector.tensor_add(
                        vt[i][:, j * 512:(j + 1) * 512], psv[j][:],
                        bv_sb[:, j * 512:(j + 1) * 512],
                    )

            # ---- attention ----
            inv_sqrt_h = float(1.0 / math.sqrt(H))
            for m in range(ST):
                ms = slice(m * P, (m + 1) * P)
                E = work.tile([P, S], bf, tag="E", name="E")
                zacc = stats.tile([P, NCH], f32, tag="zacc", name="zacc")
                sc = [psp.tile([P, 512], f32, tag="ps", name="sc", bufs=6)
                      for _ in range(NCH)]
                for k in range(KT):
                    for j in range(NCH):
                        nc.tensor.matmul(
                            sc[j][:],
                            lhsT=qT[k][:, ms],
                            rhs=kTt[k][:, j * 512:(j + 1) * 512],
                            start=(k == 0), stop=(k == KT - 1),
                        )
                for j in range(NCH):
                    nc.scalar.activation(
                        E[:, j * 512:(j + 1) * 512], sc[j][:], Exp,
                        scale=inv_sqrt_h, accum_out=zacc[:, j:j + 1],
                    )
                z = stats.tile([P, 1], f32, tag="z", name="z")
                nc.vector.reduce_sum(z[:], zacc[:], axis=mybir.AxisListType.X)
                r = stats.tile([P, 1], f32, tag="r", name="r")
                nc.vector.reciprocal(r[:], z[:])

                pr = work.tile([P, S], f32, tag="pr", name="pr")
                nc.vector.tensor_scalar_mul(pr[:], E[:], r[:])
                nc.gpsimd.dma_start(out=probs_d.ap()[ms, :], in_=pr[:])

                ET = work.tile([P, ST, P], bf, tag="ET", name="ET")
                nc.sync.dma_start(out=ET[:], in_=E[:], transpose=True)

                for j in range(HCH):
                    js = slice(j * 512, (j + 1) * 512)
                    op = psp.tile([P, 512], f32, tag="op", name="op", bufs=2)
                    for k2 in range(ST):
                        nc.tensor.matmul(
                            op[:],
                            lhsT=ET[:, k2, :],
                            rhs=vt[k2][:, js],
                            start=(k2 == 0), stop=(k2 == ST - 1),
                        )
                    ot = work.tile([P, 512], f32, tag="ot", name="ot")
                    nc.vector.tensor_scalar_mul(ot[:], op[:], r[:])
                    nc.gpsimd.dma_start(out=out_d.ap()[ms, js], in_=ot[:])

    nc.compile()
    return nc


def _prep_shared(Wq, bq, Wk, bk, Wv, bv, Wl):
    mask = _locality_mask(H, SQ)
    Wqm = Wq.astype(np.float32) * mask
    Wkm = Wk.astype(np.float32) * mask
    Wvm = Wv.astype(np.float32) * mask
    Wlm = Wl.astype(np.float32) * mask
    Wq_eff = Wlm @ Wqm
    bq_eff = Wlm @ bq.astype(np.float32)

    sup1, sup2 = _mask_supports()
    nbr_kv = _block_lists(sup1)
    nbr_q = _block_lists(sup2)

    def pack(WT, nbr):
        # WT: [h, h'] = W.T. Pack nonzero blocks: out[k, :, b, :] = WT block
        # (k, nbr[k][b]).
        nb = len(nbr[0])
        outp = np.zeros((KT, P, nb, P), dtype=np.float32)
        for k in range(KT):
            for b, m in enumerate(nbr[k]):
                outp[k, :, b, :] = WT[k * P:(k + 1) * P, m * P:(m + 1) * P]
        return outp.astype(BF16)

    return {
        "wqP": pack(np.ascontiguousarray(Wq_eff.T), nbr_q),
        "wkP": pack(np.ascontiguousarray(Wkm.T), nbr_kv),
        "wvP": pack(np.ascontiguousarray(Wvm.T), nbr_kv),
        "bq": np.ascontiguousarray(bq_eff.reshape(KT, P).T).astype(np.float32),
        "bk": np.ascontiguousarray(bk.astype(np.float32).reshape(KT, P).T),
        "bv": bv.astype(np.float32),
    }


def _make_in_maps(inputs):
    x = np.asarray(inputs["x"])
    shared = _prep_shared(
        np.asarray(inputs["Wq"]), np.asarray(inputs["bq"]),
        np.asarray(inputs["Wk"]), np.asarray(inputs["bk"]),
        np.asarray(inputs["Wv"]), np.asarray(inputs["bv"]),
        np.asarray(inputs["Wl"]),
    )
    in_maps = []
    for b in range(N_CORES):
        m = dict(shared)
        m["x"] = np.ascontiguousarray(x[b]).astype(BF16)
        in_maps.append(m)
    return in_maps


def _get_program():
    nc = _cache.get("nc")
    if nc is None:
        nc = _build_program()
        _cache["nc"] = nc
    return nc


def _run(inputs, trace=False, tmpdir=None):
    nc = _get_program()
    in_maps = _make_in_maps(inputs)
    res = bass_utils.run_bass_kernel_spmd(
        nc, in_maps, core_ids=list(range(N_CORES)), trace=trace, tmpdir=tmpdir,
    )
    out = np.stack([res.results[b]["out"] for b in range(N_CORES)])
    probs = np.stack([res.results[b]["probs"] for b in range(N_CORES)])
    return (out, probs), res


def kernel(**inputs):
    (out, probs), _ = _run(inputs)
    return out, probs


# revision 9
# speedup vs baseline: 1.3783x; 1.0743x over previous
"""Trainium2 Bass kernel for nn_AttentionHead (sparse/locally-connected attention).

Computation (per batch b):
    q = x @ (Wl*mask @ Wq*mask).T + (Wl*mask) @ bq        [S, H]
    k = x @ (Wk*mask).T + bk                              [S, H]
    v = x @ (Wv*mask).T + bv                              [S, H]
    scores = q @ k.T / sqrt(H)                            [S, S]
    probs  = softmax(scores, axis=-1)
    out    = probs @ v                                    [S, H]

Sharding: data-parallel over batch — core b computes batch b entirely
(weights replicated, no collectives).

The locality mask couples only units within Chebyshev distance 2 on a
32x32 grid (wrap-around), so at 128-row tile granularity (4 grid rows)
every masked weight matrix is block-tridiagonal (circulant): block
(I, J) is nonzero only for J in {I-1, I, I+1} mod 8. The folded
Wl*mask @ Wq*mask reaches +-4 grid rows = +-1 block, so it is block-
tridiagonal too. Projections therefore skip 5 of 8 contraction blocks.

On-core dataflow (all matmuls bf16 inputs, fp32 PSUM accumulate):
    xT   <- DMA-xbar-transpose(x)                 [h-part, s-free]
    qT,kT <- block-sparse W-stationary matmuls    [h'-part, s-free]
    v    <- xT-stationary block-sparse matmuls    [t-part, h-free]
    per 128-row block m:
        scores -> PSUM, ACT exp(+rowsum) -> E (bf16)
        probs  = E * (1/Z)  (DVE, per-partition scalar) -> DRAM
        ET     <- one DMA-xbar-transpose of E
        out    = (ET.T @ v) * (1/Z) -> DRAM
"""

import math

import ml_dtypes
import numpy as np

import concourse.bass as bass
import concourse.mybir as mybir
import concourse.tile as tile
from concourse import bacc, bass_utils

BF16 = ml_dtypes.bfloat16

B, S, H = 8, 2048, 1024
SQ = 5
P = 128
KT = H // P        # 8 feature tiles
ST = S // P        # 16 sequence blocks
NCH = S // 512     # 4 512-chunks over s/t
HCH = H // 512     # 2 512-chunks over h
N_CORES = 8

_cache = {}


def _locality_mask(hidden_size: int, width: int) -> np.ndarray:
    side = int(round(math.sqrt(hidden_size)))
    assert side * side == hidden_size
    r = np.arange(hidden_size) // side
    c = np.arange(hidden_size) % side
    dr = np.abs(r[:, None] - r[None, :])
    dc = np.abs(c[:, None] - c[None, :])
    dr = np.minimum(dr, side - dr)
    dc = np.minimum(dc, side - dc)
    half = width // 2
    return ((dr <= half) & (dc <= half)).astype(np.float32)


def _block_lists(support: np.ndarray):
    """support: [H, H] bool-ish. Returns blists[i] = sorted js with any
    nonzero in 128-block (i, j)."""
    blk = support.reshape(KT, P, KT, P).any(axis=(1, 3))
    return [sorted(np.nonzero(blk[i])[0].tolist()) for i in range(KT)]


def _mask_supports():
    mask = _locality_mask(H, SQ)
    sup1 = mask > 0                       # support of Wk', Wv' (symmetric)
    sup2 = (mask @ mask) > 0              # support of Wl'@Wq'
    return sup1, sup2


def _build_program():
    f32 = mybir.dt.float32
    bf = mybir.dt.bfloat16
    PSUM = bass.MemorySpace.PSUM
    Ident = mybir.ActivationFunctionType.Identity
    Exp = mybir.ActivationFunctionType.Exp

    sup1, sup2 = _mask_supports()
    # For W.T block (k, m): nonzero iff W[m-block, k-block] nonzero.
    # sup is symmetric so row/col lists coincide; keep general anyway.
    nbr_kv = _block_lists(sup1)   # nbr_kv[k] = m/J blocks coupled to k
    nbr_q = _block_lists(sup2)
    NB1 = max(len(l) for l in nbr_kv)
    NBQ = max(len(l) for l in nbr_q)
    assert all(len(l) == NB1 for l in nbr_kv)
    assert all(len(l) == NBQ for l in nbr_q)

    nc = bacc.Bacc("TRN2", target_bir_lowering=False, debug=False)

    xt_d = nc.dram_tensor("xT", [H, S], bf, kind="ExternalInput")
    # packed nonzero 128x128 blocks of W.T, per feature tile k
    wq_d = nc.dram_tensor("wqP", [KT, P, NBQ, P], bf, kind="ExternalInput")
    wk_d = nc.dram_tensor("wkP", [KT, P, NB1, P], bf, kind="ExternalInput")
    wv_d = nc.dram_tensor("wvP", [KT, P, NB1, P], bf, kind="ExternalInput")
    bq_d = nc.dram_tensor("bq", [P, KT], f32, kind="ExternalInput")
    bk_d = nc.dram_tensor("bk", [P, KT], f32, kind="ExternalInput")
    bv_d = nc.dram_tensor("bv", [H], f32, kind="ExternalInput")
    out_d = nc.dram_tensor("out", [S, H], f32, kind="ExternalOutput")
    probs_d = nc.dram_tensor("probs", [S, S], f32, kind="ExternalOutput")

    # m-order so each block's xT neighbors are among the earliest loads
    m_order = list(range(1, KT)) + [0]

    with tile.TileContext(nc) as tc:
        with (
            tc.tile_pool(name="sb", bufs=1) as sb,
            tc.tile_pool(name="work", bufs=2) as work,
            tc.tile_pool(name="stats", bufs=4) as stats,
            tc.tile_pool(name="ps", bufs=1, space=PSUM) as psp,
        ):
            # persistent activations
            qT = [sb.tile([P, S], bf, tag=f"qT{k}", name=f"qT{k}") for k in range(KT)]
            kTt = [sb.tile([P, S], bf, tag=f"kT{k}", name=f"kT{k}") for k in range(KT)]
            vt = [sb.tile([P, H], bf, tag=f"v{i}", name=f"v{i}") for i in range(ST)]
            # packed weights + biases + xT
            wq_sb = [sb.tile([P, NBQ, P], bf, tag=f"wq{k}", name=f"wq{k}") for k in range(KT)]
            wk_sb = [sb.tile([P, NB1, P], bf, tag=f"wk{k}", name=f"wk{k}") for k in range(KT)]
            wv_sb = [sb.tile([P, NB1, P], bf, tag=f"wv{k}", name=f"wv{k}") for k in range(KT)]
            xT = [sb.tile([P, S], bf, tag=f"xT{k}", name=f"xT{k}") for k in range(KT)]
            bq_sb = sb.tile([P, KT], f32, tag="bq")
            bk_sb = sb.tile([P, KT], f32, tag="bk")
            bv_sb = sb.tile([P, H], f32, tag="bv")

            for k in range(KT):
                nc.sync.dma_start(out=xT[k][:], in_=xt_d.ap()[k * P:(k + 1) * P, :])
            for k in range(KT):
                nc.gpsimd.dma_start(out=wk_sb[k][:], in_=wk_d.ap()[k])
                nc.gpsimd.dma_start(out=wq_sb[k][:], in_=wq_d.ap()[k])
                nc.gpsimd.dma_start(out=wv_sb[k][:], in_=wv_d.ap()[k])
            nc.gpsimd.dma_start(out=bq_sb[:], in_=bq_d.ap())
            nc.gpsimd.dma_start(out=bk_sb[:], in_=bk_d.ap())
            bv_ap = bv_d.ap()
            bv_bcast = bass.AP(tensor=bv_ap.tensor, offset=bv_ap.offset,
                               ap=[[0, P]] + list(bv_ap.ap))
            nc.gpsimd.dma_start(out=bv_sb[:], in_=bv_bcast)

            # ---- kT, qT: block-sparse, weight-stationary ----
            for w_sb, b_sb, dstT, nbr in (
                (wk_sb, bk_sb, kTt, nbr_kv),
                (wq_sb, bq_sb, qT, nbr_q),
            ):
                for m in m_order:
                    klist = [k for k in range(KT) if m in nbr[k]]
                    ps = [psp.tile([P, 512], f32, tag="ps", name="ps", bufs=5)
                          for _ in range(NCH)]
                    for ki, k in enumerate(klist):
                        bidx = nbr[k].index(m)
                        for j in range(NCH):
                            nc.tensor.matmul(
                                ps[j][:],
                                lhsT=w_sb[k][:, bidx, :],
                                rhs=xT[k][:, j * 512:(j + 1) * 512],
                                start=(ki == 0), stop=(ki == len(klist) - 1),
                            )
                    for j in range(NCH):
                        nc.scalar.activation(
                            dstT[m][:, j * 512:(j + 1) * 512], ps[j][:], Ident,
                            bias=b_sb[:, m:m + 1],
                        )

            # ---- v: xT-stationary, block-sparse over output chunks ----
            for i in range(ST):
                psv = [psp.tile([P, 512], f32, tag="ps", name="psv", bufs=5)
                       for _ in range(HCH)]
                for J in range(KT):
                    contribs = [kk for kk in range(KT) if J in nbr_kv[kk]]
                    for ci, k in enumerate(contribs):
                        bidx = nbr_kv[k].index(J)
                        nc.tensor.matmul(
                            psv[J // 4][:, (J % 4) * P:(J % 4 + 1) * P],
                            lhsT=xT[k][:, i * P:(i + 1) * P],
                            rhs=wv_sb[k][:, bidx, :],
                            start=(ci == 0), stop=(ci == len(contribs) - 1),
                        )
                for j in range(HCH):
                    nc.vector.tensor_add(
                        vt[i][:, j * 512:(j + 1) * 512], psv[j][:],
                        bv_sb[:, j * 512:(j + 1) * 512],
                    )

            # ---- attention ----
            inv_sqrt_h = float(1.0 / math.sqrt(H))
            for m in range(ST):
                ms = slice(m * P, (m + 1) * P)
                E = work.tile([P, S], bf, tag="E", name="E")
                zacc = stats.tile([P, NCH], f32, tag="zacc", name="zacc")
                sc = [psp.tile([P, 512], f32, tag="ps", name="sc", bufs=5)
                      for _ in range(NCH)]
                for k in range(KT):
                    for j in range(NCH):
                        nc.tensor.matmul(
                            sc[j][:],
                            lhsT=qT[k][:, ms],
                            rhs=kTt[k][:, j * 512:(j + 1) * 512],
                            start=(k == 0), stop=(k == KT - 1),
                        )
                for j in range(NCH):
                    nc.scalar.activation(
                        E[:, j * 512:(j + 1) * 512], sc[j][:], Exp,
                        scale=inv_sqrt_h, accum_out=zacc[:, j:j + 1],
                    )
                z = stats.tile([P, 1], f32, tag="z", name="z")
                nc.vector.reduce_sum(z[:], zacc[:], axis=mybir.AxisListType.X)
                r = stats.tile([P, 1], f32, tag="r", name="r")
                nc.vector.reciprocal(r[:], z[:])

                pr = work.tile([P, S], f32, tag="pr", name="pr")
                nc.vector.tensor_scalar_mul(pr[:], E[:], r[:])
                nc.gpsimd.dma_start(out=probs_d.ap()[ms, :], in_=pr[:])

                ET = work.tile([P, ST, P], bf, tag="ET", name="ET")
                nc.sync.dma_start(out=ET[:], in_=E[:], transpose=True)

                op = [psp.tile([P, 512], f32, tag="op", name="op", bufs=3)
                      for _ in range(HCH)]
                for k2 in range(ST):
                    for j in range(HCH):
                        nc.tensor.matmul(
                            op[j][:],
                            lhsT=ET[:, k2, :],
                            rhs=vt[k2][:, j * 512:(j + 1) * 512],
                            start=(k2 == 0), stop=(k2 == ST - 1),
                        )
                for j in range(HCH):
                    js = slice(j * 512, (j + 1) * 512)
                    ot = work.tile([P, 512], f32, tag="ot", name="ot")
                    nc.vector.tensor_scalar_mul(ot[:], op[j][:], r[:])
                    nc.gpsimd.dma_start(out=out_d.ap()[ms, js], in_=ot[:])

    nc.compile()
    return nc


def _prep_shared(Wq, bq, Wk, bk, Wv, bv, Wl):
    mask = _locality_mask(H, SQ)
    Wqm = Wq.astype(np.float32) * mask
    Wkm = Wk.astype(np.float32) * mask
    Wvm = Wv.astype(np.float32) * mask
    Wlm = Wl.astype(np.float32) * mask
    Wq_eff = Wlm @ Wqm
    bq_eff = Wlm @ bq.astype(np.float32)

    sup1, sup2 = _mask_supports()
    nbr_kv = _block_lists(sup1)
    nbr_q = _block_lists(sup2)

    def pack(WT, nbr):
        # WT: [h, h'] = W.T. Pack nonzero blocks: out[k, :, b, :] = WT block
        # (k, nbr[k][b]).
        nb = len(nbr[0])
        outp = np.zeros((KT, P, nb, P), dtype=np.float32)
        for k in range(KT):
            for b, m in enumerate(nbr[k]):
                outp[k, :, b, :] = WT[k * P:(k + 1) * P, m * P:(m + 1) * P]
        return outp.astype(BF16)

    return {
        "wqP": pack(np.ascontiguousarray(Wq_eff.T), nbr_q),
        "wkP": pack(np.ascontiguousarray(Wkm.T), nbr_kv),
        "wvP": pack(np.ascontiguousarray(Wvm.T), nbr_kv),
        "bq": np.ascontiguousarray(bq_eff.reshape(KT, P).T).astype(np.float32),
        "bk": np.ascontiguousarray(bk.astype(np.float32).reshape(KT, P).T),
        "bv": bv.astype(np.float32),
    }


def _make_in_maps(inputs):
    x = np.asarray(inputs["x"])
    shared = _prep_shared(
        np.asarray(inputs["Wq"]), np.asarray(inputs["bq"]),
        np.asarray(inputs["Wk"]), np.asarray(inputs["bk"]),
        np.asarray(inputs["Wv"]), np.asarray(inputs["bv"]),
        np.asarray(inputs["Wl"]),
    )
    in_maps = []
    for b in range(N_CORES):
        m = dict(shared)
        m["xT"] = np.ascontiguousarray(x[b].astype(BF16).T)
        in_maps.append(m)
    return in_maps


def _get_program():
    nc = _cache.get("nc")
    if nc is None:
        nc = _build_program()
        _cache["nc"] = nc
    return nc


def _run(inputs, trace=False, tmpdir=None):
    nc = _get_program()
    in_maps = _make_in_maps(inputs)
    res = bass_utils.run_bass_kernel_spmd(
        nc, in_maps, core_ids=list(range(N_CORES)), trace=trace, tmpdir=tmpdir,
    )
    out = np.stack([res.results[b]["out"] for b in range(N_CORES)])
    probs = np.stack([res.results[b]["probs"] for b in range(N_CORES)])
    return (out, probs), res


def kernel(**inputs):
    (out, probs), _ = _run(inputs)
    return out, probs


# revision 14
# speedup vs baseline: 1.5916x; 1.1548x over previous
"""Trainium2 Bass kernel for nn_AttentionHead (sparse/locally-connected attention).

Computation (per batch b):
    q = x @ (Wl*mask @ Wq*mask).T + (Wl*mask) @ bq        [S, H]
    k = x @ (Wk*mask).T + bk                              [S, H]
    v = x @ (Wv*mask).T + bv                              [S, H]
    scores = q @ k.T / sqrt(H)                            [S, S]
    probs  = softmax(scores, axis=-1)
    out    = probs @ v                                    [S, H]

Sharding: data-parallel over batch — core b computes batch b entirely
(weights replicated, no collectives).

The locality mask couples only units within Chebyshev distance 2 on a
32x32 grid (wrap-around), so at 128-row tile granularity (4 grid rows)
every masked weight matrix is block-tridiagonal (circulant): block
(I, J) is nonzero only for J in {I-1, I, I+1} mod 8. The folded
Wl*mask @ Wq*mask reaches +-4 grid rows = +-1 block, so it is block-
tridiagonal too. Projections therefore skip 5 of 8 contraction blocks.

On-core dataflow (all matmuls bf16 inputs, fp32 PSUM accumulate):
    xT   <- DMA-xbar-transpose(x)                 [h-part, s-free]
    qT,kT <- block-sparse W-stationary matmuls    [h'-part, s-free]
    v    <- xT-stationary block-sparse matmuls    [t-part, h-free]
    per 128-row block m:
        scores -> PSUM, ACT exp(+rowsum) -> E (bf16)
        probs  = E * (1/Z)  (DVE, per-partition scalar) -> DRAM
        ET     <- one DMA-xbar-transpose of E
        out    = (ET.T @ v) * (1/Z) -> DRAM
"""

import math

import ml_dtypes
import numpy as np

import concourse.bass as bass
import concourse.mybir as mybir
import concourse.tile as tile
from concourse import bacc, bass_utils

BF16 = ml_dtypes.bfloat16

B, S, H = 8, 2048, 1024
SQ = 5
P = 128
KT = H // P        # 8 feature tiles
ST = S // P        # 16 sequence blocks
NCH = S // 512     # 4 512-chunks over s/t
HCH = H // 512     # 2 512-chunks over h
N_CORES = 8

_cache = {}


def _locality_mask(hidden_size: int, width: int) -> np.ndarray:
    side = int(round(math.sqrt(hidden_size)))
    assert side * side == hidden_size
    r = np.arange(hidden_size) // side
    c = np.arange(hidden_size) % side
    dr = np.abs(r[:, None] - r[None, :])
    dc = np.abs(c[:, None] - c[None, :])
    dr = np.minimum(dr, side - dr)
    dc = np.minimum(dc, side - dc)
    half = width // 2
    return ((dr <= half) & (dc <= half)).astype(np.float32)


def _block_lists(support: np.ndarray):
    """support: [H, H] bool-ish. Returns blists[i] = sorted js with any
    nonzero in 128-block (i, j)."""
    blk = support.reshape(KT, P, KT, P).any(axis=(1, 3))
    return [sorted(np.nonzero(blk[i])[0].tolist()) for i in range(KT)]


def _mask_supports():
    mask = _locality_mask(H, SQ)
    sup1 = mask > 0                       # support of Wk', Wv' (symmetric)
    sup2 = (mask @ mask) > 0              # support of Wl'@Wq'
    return sup1, sup2


def _build_program():
    f32 = mybir.dt.float32
    bf = mybir.dt.bfloat16
    PSUM = bass.MemorySpace.PSUM
    Ident = mybir.ActivationFunctionType.Identity
    Exp = mybir.ActivationFunctionType.Exp

    sup1, sup2 = _mask_supports()
    # For W.T block (k, m): nonzero iff W[m-block, k-block] nonzero.
    # sup is symmetric so row/col lists coincide; keep general anyway.
    nbr_kv = _block_lists(sup1)   # nbr_kv[k] = m/J blocks coupled to k
    nbr_q = _block_lists(sup2)
    NB1 = max(len(l) for l in nbr_kv)
    NBQ = max(len(l) for l in nbr_q)
    assert all(len(l) == NB1 for l in nbr_kv)
    assert all(len(l) == NBQ for l in nbr_q)

    nc = bacc.Bacc("TRN2", target_bir_lowering=False, debug=False)

    xt_d = nc.dram_tensor("xT", [H, S], bf, kind="ExternalInput")
    # packed nonzero 128x128 blocks of W.T, partition-major for one-shot DMA:
    # [p, k, b, c] = W.T[k*128+p, nbr[k][b]*128+c]
    wq_d = nc.dram_tensor("wqP", [P, KT, NBQ, P], bf, kind="ExternalInput")
    wk_d = nc.dram_tensor("wkP", [P, KT, NB1, P], bf, kind="ExternalInput")
    wv_d = nc.dram_tensor("wvP", [P, KT, NB1, P], bf, kind="ExternalInput")
    bq_d = nc.dram_tensor("bq", [P, KT], f32, kind="ExternalInput")
    bk_d = nc.dram_tensor("bk", [P, KT], f32, kind="ExternalInput")
    bv_d = nc.dram_tensor("bv", [H], f32, kind="ExternalInput")
    out_d = nc.dram_tensor("out", [S, H], f32, kind="ExternalOutput")
    probs_d = nc.dram_tensor("probs", [S, S], f32, kind="ExternalOutput")

    # m-order so each block's xT neighbors are among the earliest loads
    m_order = list(range(1, KT)) + [0]

    with tile.TileContext(nc) as tc:
        with (
            tc.tile_pool(name="sb", bufs=1) as sb,
            tc.tile_pool(name="work", bufs=2) as work,
            tc.tile_pool(name="stats", bufs=4) as stats,
            tc.tile_pool(name="ps", bufs=1, space=PSUM) as psp,
        ):
            # persistent activations
            qT = [sb.tile([P, S], bf, tag=f"qT{k}", name=f"qT{k}") for k in range(KT)]
            kTt = [sb.tile([P, S], bf, tag=f"kT{k}", name=f"kT{k}") for k in range(KT)]
            vt = [sb.tile([P, H], bf, tag=f"v{i}", name=f"v{i}") for i in range(ST)]
            # packed weights + biases + xT
            wq_all = sb.tile([P, KT, NBQ, P], bf, tag="wq", name="wq_all")
            wk_all = sb.tile([P, KT, NB1, P], bf, tag="wk", name="wk_all")
            wv_all = sb.tile([P, KT, NB1, P], bf, tag="wv", name="wv_all")
            wq_sb = [wq_all[:, k] for k in range(KT)]
            wk_sb = [wk_all[:, k] for k in range(KT)]
            wv_sb = [wv_all[:, k] for k in range(KT)]
            xT = [sb.tile([P, S], bf, tag=f"xT{k}", name=f"xT{k}") for k in range(KT)]
            bq_sb = sb.tile([P, KT], f32, tag="bq")
            bk_sb = sb.tile([P, KT], f32, tag="bk")
            bv_sb = sb.tile([P, H], f32, tag="bv")

            # dummy exp up front so the ACT table load happens before any
            # real dependency chain (it otherwise lands behind the input DMA
            # queue and stalls every downstream activation)
            dummy = stats.tile([P, 1], f32, tag="dmy", name="dummy")
            nc.vector.memset(dummy[:], 0.0)
            nc.scalar.activation(dummy[:], dummy[:], Exp)

            for k in range(KT):
                nc.sync.dma_start(out=xT[k][:], in_=xt_d.ap()[k * P:(k + 1) * P, :])
            nc.gpsimd.dma_start(out=wk_all[:], in_=wk_d.ap())
            nc.gpsimd.dma_start(out=wq_all[:], in_=wq_d.ap())
            nc.gpsimd.dma_start(out=wv_all[:], in_=wv_d.ap())
            nc.gpsimd.dma_start(out=bq_sb[:], in_=bq_d.ap())
            nc.gpsimd.dma_start(out=bk_sb[:], in_=bk_d.ap())
            bv_ap = bv_d.ap()
            bv_bcast = bass.AP(tensor=bv_ap.tensor, offset=bv_ap.offset,
                               ap=[[0, P]] + list(bv_ap.ap))
            nc.gpsimd.dma_start(out=bv_sb[:], in_=bv_bcast)

            # ---- kT, qT: block-sparse, weight-stationary ----
            for w_sb, b_sb, dstT, nbr in (
                (wk_sb, bk_sb, kTt, nbr_kv),
                (wq_sb, bq_sb, qT, nbr_q),
            ):
                for m in m_order:
                    klist = [k for k in range(KT) if m in nbr[k]]
                    ps = [psp.tile([P, 512], f32, tag="ps", name="ps", bufs=5)
                          for _ in range(NCH)]
                    for ki, k in enumerate(klist):
                        bidx = nbr[k].index(m)
                        for j in range(NCH):
                            nc.tensor.matmul(
                                ps[j][:],
                                lhsT=w_sb[k][:, bidx, :],
                                rhs=xT[k][:, j * 512:(j + 1) * 512],
                                start=(ki == 0), stop=(ki == len(klist) - 1),
                            )
                    for j in range(NCH):
                        nc.scalar.activation(
                            dstT[m][:, j * 512:(j + 1) * 512], ps[j][:], Ident,
                            bias=b_sb[:, m:m + 1],
                        )

            # ---- v: xT-stationary, block-sparse over output chunks ----
            for i in range(ST):
                psv = [psp.tile([P, 512], f32, tag="ps", name="psv", bufs=5)
                       for _ in range(HCH)]
                for J in range(KT):
                    contribs = [kk for kk in range(KT) if J in nbr_kv[kk]]
                    for ci, k in enumerate(contribs):
                        bidx = nbr_kv[k].index(J)
                        nc.tensor.matmul(
                            psv[J // 4][:, (J % 4) * P:(J % 4 + 1) * P],
                            lhsT=xT[k][:, i * P:(i + 1) * P],
                            rhs=wv_sb[k][:, bidx, :],
                            start=(ci == 0), stop=(ci == len(contribs) - 1),
                        )
                for j in range(HCH):
                    nc.vector.tensor_add(
                        vt[i][:, j * 512:(j + 1) * 512], psv[j][:],
                        bv_sb[:, j * 512:(j + 1) * 512],
                    )

            # ---- attention ----
            inv_sqrt_h = float(1.0 / math.sqrt(H))
            for m in range(ST):
                ms = slice(m * P, (m + 1) * P)
                E = work.tile([P, S], bf, tag="E", name="E")
                ET = work.tile([P, ST, P], bf, tag="ET", name="ET", bufs=3)
                zacc = stats.tile([P, NCH], f32, tag="zacc", name="zacc")
                for j in range(NCH):
                    js = slice(j * 512, (j + 1) * 512)
                    sc = psp.tile([P, 512], f32, tag="ps", name="sc", bufs=5)
                    for k in range(KT):
                        nc.tensor.matmul(
                            sc[:],
                            lhsT=qT[k][:, ms],
                            rhs=kTt[k][:, js],
                            start=(k == 0), stop=(k == KT - 1),
                        )
                    nc.scalar.activation(
                        E[:, js], sc[:], Exp,
                        scale=inv_sqrt_h, accum_out=zacc[:, j:j + 1],
                    )
                    nc.sync.dma_start(out=ET[:, 4 * j:4 * (j + 1), :],
                                      in_=E[:, js], transpose=True)
                z = stats.tile([P, 1], f32, tag="z", name="z")
                nc.vector.reduce_sum(z[:], zacc[:], axis=mybir.AxisListType.X)
                r = stats.tile([P, 1], f32, tag="r", name="r")
                nc.vector.reciprocal(r[:], z[:])

                pr = work.tile([P, S], f32, tag="pr", name="pr")
                nc.vector.tensor_scalar_mul(pr[:], E[:], r[:])
                nc.gpsimd.dma_start(out=probs_d.ap()[ms, :], in_=pr[:])

                op = [psp.tile([P, 512], f32, tag="op", name="op", bufs=3)
                      for _ in range(HCH)]
                for k2 in range(ST):
                    for j in range(HCH):
                        nc.tensor.matmul(
                            op[j][:],
                            lhsT=ET[:, k2, :],
                            rhs=vt[k2][:, j * 512:(j + 1) * 512],
                            start=(k2 == 0), stop=(k2 == ST - 1),
                        )
                for j in range(HCH):
                    js = slice(j * 512, (j + 1) * 512)
                    ot = work.tile([P, 512], f32, tag="ot", name="ot")
                    nc.vector.tensor_scalar_mul(ot[:], op[j][:], r[:])
                    nc.gpsimd.dma_start(out=out_d.ap()[ms, js], in_=ot[:])

    nc.compile()
    return nc


def _prep_shared(Wq, bq, Wk, bk, Wv, bv, Wl):
    mask = _locality_mask(H, SQ)
    Wqm = Wq.astype(np.float32) * mask
    Wkm = Wk.astype(np.float32) * mask
    Wvm = Wv.astype(np.float32) * mask
    Wlm = Wl.astype(np.float32) * mask
    Wq_eff = Wlm @ Wqm
    bq_eff = Wlm @ bq.astype(np.float32)

    sup1, sup2 = _mask_supports()
    nbr_kv = _block_lists(sup1)
    nbr_q = _block_lists(sup2)

    def pack(WT, nbr):
        # WT: [h, h'] = W.T. Partition-major pack of nonzero blocks:
        # out[p, k, b, :] = WT[k*128+p, nbr[k][b]*128 : +128].
        nb = len(nbr[0])
        outp = np.zeros((P, KT, nb, P), dtype=np.float32)
        for k in range(KT):
            for b, m in enumerate(nbr[k]):
                outp[:, k, b, :] = WT[k * P:(k + 1) * P, m * P:(m + 1) * P]
        return np.ascontiguousarray(outp).astype(BF16)

    return {
        "wqP": pack(np.ascontiguousarray(Wq_eff.T), nbr_q),
        "wkP": pack(np.ascontiguousarray(Wkm.T), nbr_kv),
        "wvP": pack(np.ascontiguousarray(Wvm.T), nbr_kv),
        "bq": np.ascontiguousarray(bq_eff.reshape(KT, P).T).astype(np.float32),
        "bk": np.ascontiguousarray(bk.astype(np.float32).reshape(KT, P).T),
        "bv": bv.astype(np.float32),
    }


def _make_in_maps(inputs):
    x = np.asarray(inputs["x"])
    shared = _prep_shared(
        np.asarray(inputs["Wq"]), np.asarray(inputs["bq"]),
        np.asarray(inputs["Wk"]), np.asarray(inputs["bk"]),
        np.asarray(inputs["Wv"]), np.asarray(inputs["bv"]),
        np.asarray(inputs["Wl"]),
    )
    in_maps = []
    for b in range(N_CORES):
        m = dict(shared)
        m["xT"] = np.ascontiguousarray(x[b].astype(BF16).T)
        in_maps.append(m)
    return in_maps


def _get_program():
    nc = _cache.get("nc")
    if nc is None:
        nc = _build_program()
        _cache["nc"] = nc
    return nc


def _run(inputs, trace=False, tmpdir=None):
    nc = _get_program()
    in_maps = _make_in_maps(inputs)
    res = bass_utils.run_bass_kernel_spmd(
        nc, in_maps, core_ids=list(range(N_CORES)), trace=trace, tmpdir=tmpdir,
    )
    out = np.stack([res.results[b]["out"] for b in range(N_CORES)])
    probs = np.stack([res.results[b]["probs"] for b in range(N_CORES)])
    return (out, probs), res


def kernel(**inputs):
    (out, probs), _ = _run(inputs)
    return out, probs
